# revision 6
# baseline (speedup 1.0000x reference)
"""Trainium2 Bass kernel for nn_CVX_Reasoning_Engine.

MLP (16384x512 -> 512 -> 256 -> 128 -> 64 -> 256) with LeakyReLU(0.2),
followed by a closed-form per-object/axis QP solve.

Strategy:
- Pure data parallel over 8 NeuronCores (2048 batch rows each).
- Host-side prep: fold `bounds` contribution of the concat into layer-1
  bias; transpose z so activations flow feature-major on-chip (no
  device transposes between layers); append the layer-5 bias as an
  extra ones-row of the last activation (K=65 matmul) so layer 5 exits
  batch-major, ready for the elementwise QP and a contiguous store.
- Matmuls run in float32r (full-rate fp32 mode, TF32-like precision).
- QP closed form without branches:
    x0 = max(pp, lo); g0 = max(pg, 1); s0 = x0 + g0
    w  = min(s0, hi)
    xl = clip(0.5*(pp - pg + hi), lo, hi-1)
    x  = min(x0, xl)          # provably equals the reference select
"""

import numpy as np

BS, Z, NOBJ = 16384, 512, 64
NCORES = 8
BSC = BS // NCORES            # 2048 batch rows per core
NCHUNK = 512                  # batch columns per matmul chunk
NCHUNKS = BSC // NCHUNK       # 4
P = 128
NSTG = 2                      # QP stagings per core
SUBS_PER_STG = (BSC // P) // NSTG   # 8 subtiles of 128 batch rows each

_cache = {}


def _build(b0, b1, b2, b3):
    import concourse.tile as tile
    from concourse import bacc, mybir

    f32 = mybir.dt.float32
    f32r = mybir.dt.float32r
    AF = mybir.ActivationFunctionType
    Alu = mybir.AluOpType

    nc = bacc.Bacc("TRN2", target_bir_lowering=False, debug=False,
                   num_devices=NCORES)

    zt_d = nc.dram_tensor("zt", (Z, BSC), f32r, kind="ExternalInput").ap()
    w1_d = nc.dram_tensor("w1", (512, 512), f32r, kind="ExternalInput").ap()
    w2_d = nc.dram_tensor("w2", (512, 256), f32r, kind="ExternalInput").ap()
    w3_d = nc.dram_tensor("w3", (256, 128), f32r, kind="ExternalInput").ap()
    w4_d = nc.dram_tensor("w4", (128, 64), f32r, kind="ExternalInput").ap()
    w5_d = nc.dram_tensor("w5", (65, 256), f32r, kind="ExternalInput").ap()
    b1_d = nc.dram_tensor("b1", (4, 128), f32, kind="ExternalInput").ap()
    b2_d = nc.dram_tensor("b2", (2, 128), f32, kind="ExternalInput").ap()
    b3_d = nc.dram_tensor("b3", (128, 1), f32, kind="ExternalInput").ap()
    b4_d = nc.dram_tensor("b4", (64, 1), f32, kind="ExternalInput").ap()
    o_d = nc.dram_tensor("o", (BSC, 256), f32, kind="ExternalOutput").ap()

    with tile.TileContext(nc) as tc:
        with (
            tc.tile_pool(name="wp", bufs=1) as wp,
            tc.tile_pool(name="zp", bufs=3) as zp,
            tc.tile_pool(name="hp", bufs=2) as hp,
            tc.tile_pool(name="stg", bufs=2) as stg,
            tc.tile_pool(name="tmp", bufs=2) as tmp,
            tc.tile_pool(name="ps", bufs=5, space="PSUM") as ps,
            tc.tile_pool(name="ps5", bufs=3, space="PSUM") as ps5p,
        ):
            # ---- resident weights & biases ----
            w1_sb = wp.tile([P, 4 * 512], f32r, tag="w1")
            nc.sync.dma_start(
                w1_sb[:].rearrange("p (k m) -> p k m", k=4),
                w1_d.rearrange("(k p) m -> p k m", p=P))
            w2_sb = wp.tile([P, 4 * 256], f32r, tag="w2")
            nc.sync.dma_start(
                w2_sb[:].rearrange("p (k m) -> p k m", k=4),
                w2_d.rearrange("(k p) m -> p k m", p=P))
            w3_sb = wp.tile([P, 2 * 128], f32r, tag="w3")
            nc.sync.dma_start(
                w3_sb[:].rearrange("p (k m) -> p k m", k=2),
                w3_d.rearrange("(k p) m -> p k m", p=P))
            w4_sb = wp.tile([P, 64], f32r, tag="w4")
            nc.sync.dma_start(w4_sb[:], w4_d)
            w5_sb = wp.tile([65, 256], f32r, tag="w5")
            nc.sync.dma_start(w5_sb[:], w5_d)
            b1_sb = wp.tile([P, 4], f32, tag="b1")
            nc.sync.dma_start(b1_sb[:], b1_d.rearrange("m p -> p m"))
            b2_sb = wp.tile([P, 2], f32, tag="b2")
            nc.sync.dma_start(b2_sb[:], b2_d.rearrange("m p -> p m"))
            b3_sb = wp.tile([P, 1], f32, tag="b3")
            nc.sync.dma_start(b3_sb[:], b3_d)
            b4_sb = wp.tile([64, 1], f32, tag="b4")
            nc.sync.dma_start(b4_sb[:], b4_d)

            for sidx in range(NSTG):
                p_sb = stg.tile([P, SUBS_PER_STG * 256], f32, tag="p")
                o_sb = stg.tile([P, SUBS_PER_STG * 256], f32, tag="o")

                for nloc in range(NCHUNKS // NSTG):
                    n = sidx * (NCHUNKS // NSTG) + nloc
                    # ---- load z chunk (feature-major) ----
                    zt_n = zp.tile([P, 4 * NCHUNK], f32r, tag="zt")
                    nc.sync.dma_start(
                        zt_n[:].rearrange("p (k c) -> p k c", k=4),
                        zt_d[:, n * NCHUNK:(n + 1) * NCHUNK]
                            .rearrange("(k p) c -> p k c", p=P))

                    # ---- L1: 512 -> 512 ----
                    h1_n = hp.tile([P, 4 * NCHUNK], f32r, tag="h1")
                    for m in range(4):
                        pst = ps.tile([P, NCHUNK], f32, tag="mm")
                        for k in range(4):
                            nc.tensor.matmul(
                                pst[:],
                                w1_sb[:, k * 512 + m * 128:k * 512 + (m + 1) * 128],
                                zt_n[:, k * NCHUNK:(k + 1) * NCHUNK],
                                start=(k == 0), stop=(k == 3))
                        nc.scalar.activation(
                            h1_n[:, m * NCHUNK:(m + 1) * NCHUNK], pst[:],
                            AF.Prelu, bias=b1_sb[:, m:m + 1], alpha=0.2)

                    # ---- L2: 512 -> 256 ----
                    h2_n = hp.tile([P, 2 * NCHUNK], f32r, tag="h2")
                    for m in range(2):
                        pst = ps.tile([P, NCHUNK], f32, tag="mm")
                        for k in range(4):
                            nc.tensor.matmul(
                                pst[:],
                                w2_sb[:, k * 256 + m * 128:k * 256 + (m + 1) * 128],
                                h1_n[:, k * NCHUNK:(k + 1) * NCHUNK],
                                start=(k == 0), stop=(k == 3))
                        nc.scalar.activation(
                            h2_n[:, m * NCHUNK:(m + 1) * NCHUNK], pst[:],
                            AF.Prelu, bias=b2_sb[:, m:m + 1], alpha=0.2)

                    # ---- L3: 256 -> 128 ----
                    h3_n = hp.tile([P, NCHUNK], f32r, tag="h3")
                    pst = ps.tile([P, NCHUNK], f32, tag="mm")
                    for k in range(2):
                        nc.tensor.matmul(
                            pst[:], w3_sb[:, k * 128:(k + 1) * 128],
                            h2_n[:, k * NCHUNK:(k + 1) * NCHUNK],
                            start=(k == 0), stop=(k == 1))
                    nc.scalar.activation(h3_n[:], pst[:], AF.Prelu,
                                         bias=b3_sb[:, 0:1], alpha=0.2)

                    # ---- L4: 128 -> 64 (plus ones row for L5 bias) ----
                    h4_n = hp.tile([65, NCHUNK], f32r, tag="h4")
                    pst = ps.tile([P, NCHUNK], f32, tag="mm")
                    nc.tensor.matmul(pst[0:64, :], w4_sb[:], h3_n[:],
                                     start=True, stop=True)
                    nc.scalar.activation(h4_n[0:64, :], pst[0:64, :], AF.Prelu,
                                         bias=b4_sb[:, 0:1], alpha=0.2)
                    nc.vector.memset(h4_n[64:65, :].bitcast(f32), 1.0)

                    # ---- L5: 65 -> 256, batch-major out ----
                    for sub in range(NCHUNK // P):
                        s_in_stg = nloc * (NCHUNK // P) + sub
                        p5 = ps5p.tile([P, 256], f32, tag="l5")
                        nc.tensor.matmul(
                            p5[:], h4_n[:, sub * P:(sub + 1) * P], w5_sb[:],
                            start=True, stop=True)
                        nc.vector.tensor_copy(
                            p_sb[:, s_in_stg * 256:(s_in_stg + 1) * 256], p5[:])

                # ---- QP solve on the staging (batch-major) ----
                S = SUBS_PER_STG
                pv = p_sb[:].rearrange("p (s o c) -> p s o c", s=S, o=NOBJ)
                ov = o_sb[:].rearrange("p (s o c) -> p s o c", s=S, o=NOBJ)
                if b0 == b1 and b2 == b3:
                    groups = [((0, 2), 2, b0, b2)]
                else:
                    groups = [((0, 2), 1, b0, b2), ((1, 3), 1, b1, b3)]
                if sidx == 0:
                    qc = wp.tile([P, 2], f32, tag="qc")
                    nc.vector.memset(qc[:, 0:1], -float(b0))
                    nc.vector.memset(qc[:, 1:2], -1.0)
                    if b0 != b1:
                        qc2 = wp.tile([P, 1], f32, tag="qc2")
                        nc.vector.memset(qc2[:, 0:1], -float(b1))
                for gi, ((cpp, cpg), cw, lo, hi) in enumerate(groups):
                    lo_bias = qc[:, 0:1] if gi == 0 else qc2[:, 0:1]
                    fd = S * NOBJ * cw
                    pp = pv[:, :, :, cpp:cpp + cw]
                    pg = pv[:, :, :, cpg:cpg + cw]

                    def tview(t):
                        return t[:].rearrange("p (s o c) -> p s o c",
                                              s=S, o=NOBJ)[:, :, :, 0:cw]

                    rx = tmp.tile([P, fd], f32, tag="rx")
                    rg = tmp.tile([P, fd], f32, tag="rg")
                    s0 = tmp.tile([P, fd], f32, tag="s0")
                    u = tmp.tile([P, fd], f32, tag="u")
                    xl = tmp.tile([P, fd], f32, tag="xl")
                    rxv, rgv, s0v, uv, xlv = map(tview, (rx, rg, s0, u, xl))
                    # rx = relu(pp - lo); rg = relu(pg - 1)
                    nc.scalar.activation(rxv, pp, AF.Relu, bias=lo_bias)
                    nc.scalar.activation(rgv, pg, AF.Relu, bias=qc[:, 1:2])
                    # s0 = (rx + (lo+1)) + rg
                    nc.vector.scalar_tensor_tensor(
                        s0v, rxv, float(lo) + 1.0, rgv, Alu.add, Alu.add)
                    # w = min(s0, hi)
                    nc.vector.tensor_scalar_min(
                        ov[:, :, :, cpg:cpg + cw], s0v, float(hi))
                    # u = (pp + hi) - pg
                    nc.vector.scalar_tensor_tensor(
                        uv, pp, float(hi), pg, Alu.add, Alu.subtract)
                    # xl = clip(0.5*u, lo, hi-1)
                    nc.vector.tensor_scalar(
                        xlv, uv, 0.5, float(lo), Alu.mult, Alu.max)
                    nc.vector.tensor_scalar_min(xlv, xlv, float(hi) - 1.0)
                    # x = min(rx + lo, xl)
                    nc.vector.scalar_tensor_tensor(
                        ov[:, :, :, cpp:cpp + cw], rxv, float(lo), xlv,
                        Alu.add, Alu.min)

                # ---- store staging -> DRAM (contiguous rows) ----
                nc.sync.dma_start(
                    o_d[sidx * SUBS_PER_STG * P:(sidx + 1) * SUBS_PER_STG * P, :]
                        .rearrange("(s p) f -> p s f", p=P),
                    o_sb[:].rearrange("p (s f) -> p s f", s=SUBS_PER_STG))

    nc.compile()
    return nc


def _get_nc(b0, b1, b2, b3):
    key = (b0, b1, b2, b3)
    if key not in _cache:
        _cache[key] = _build(b0, b1, b2, b3)
    return _cache[key]


def _prep_inputs(z, bounds, W1, c1, W2, c2, W3, c3, W4, c4, W5, c5):
    b = np.asarray(bounds, np.float32)
    W1m = np.ascontiguousarray(W1[:Z], np.float32)
    b1 = (np.asarray(c1, np.float32)
          + b @ np.asarray(W1[Z:], np.float32)).astype(np.float32)
    w5a = np.concatenate(
        [np.asarray(W5, np.float32), np.asarray(c5, np.float32)[None, :]], 0)
    zT = np.ascontiguousarray(np.asarray(z, np.float32).T)
    common = {
        "w1": W1m, "w2": np.ascontiguousarray(W2, np.float32),
        "w3": np.ascontiguousarray(W3, np.float32),
        "w4": np.ascontiguousarray(W4, np.float32),
        "w5": np.ascontiguousarray(w5a),
        "b1": b1.reshape(4, 128),
        "b2": np.asarray(c2, np.float32).reshape(2, 128),
        "b3": np.asarray(c3, np.float32).reshape(128, 1),
        "b4": np.asarray(c4, np.float32).reshape(64, 1),
    }
    in_maps = []
    for i in range(NCORES):
        m = dict(common)
        m["zt"] = np.ascontiguousarray(zT[:, i * BSC:(i + 1) * BSC])
        in_maps.append(m)
    return in_maps, (float(b[0]), float(b[1]), float(b[2]), float(b[3]))


def kernel(z, bounds, W1, c1, W2, c2, W3, c3, W4, c4, W5, c5):
    from concourse.bass_utils import run_bass_kernel_spmd

    in_maps, bvals = _prep_inputs(z, bounds, W1, c1, W2, c2, W3, c3,
                                  W4, c4, W5, c5)
    nc = _get_nc(*bvals)
    res = run_bass_kernel_spmd(nc, in_maps, core_ids=list(range(NCORES)))
    out = np.concatenate([r["o"] for r in res.results], axis=0)
    return out.reshape(BS, NOBJ, 4)


# revision 10
# speedup vs baseline: 434.9923x; 434.9923x over previous
"""Trainium2 Bass kernel for nn_CVX_Reasoning_Engine.

MLP (16384x512 -> 512 -> 256 -> 128 -> 64 -> 256) with LeakyReLU(0.2),
followed by a closed-form per-object/axis QP solve.

Strategy:
- Pure data parallel over 8 NeuronCores (2048 batch rows each).
- Host-side prep: fold `bounds` contribution of the concat into layer-1
  bias; transpose z so activations flow feature-major on-chip (no
  device transposes between layers); append the layer-5 bias as an
  extra ones-row of the last activation (K=65 matmul) so layer 5 exits
  batch-major, ready for the elementwise QP and a contiguous store.
- Matmuls run in float32r (full-rate fp32 mode, TF32-like precision).
- L1 is k-outer with per-k weight/input DMA splits so the PE starts
  after ~0.5MB of input instead of 2MB.
- Chunks of 1024 batch columns; activations are one ACT op per m-tile
  (bias + PReLU(0.2) fused, PSUM -> SBUF).
- QP closed form without branches (all on DVE, immediates baked in):
    x0 = max(pp, lo); g0 = max(pg, 1); s0 = x0 + g0
    w  = min(s0, hi)
    v  = 0.5*(pp - pg + hi)
    x  = max(min(min(x0, v), hi-1), lo)   # == reference KKT select
"""

import numpy as np

BS, Z, NOBJ = 16384, 512, 64
NCORES = 8
BSC = BS // NCORES            # 2048 batch rows per core
NCH = 1024                    # batch columns per chunk
P = 128

# packed-weight layout (per-partition float32 offsets)
_W2O, _W3O, _W4O, _W5O = 0, 1024, 1280, 1344
_B1O, _B2O, _B3O, _B4O = 1600, 1604, 1606, 1607
_WKW = 1608

_cache = {}


def _build(b0, b1, b2, b3, reps=1):
    import concourse.tile as tile
    from concourse import bacc, mybir

    f32 = mybir.dt.float32
    f32r = mybir.dt.float32r
    AF = mybir.ActivationFunctionType
    Alu = mybir.AluOpType

    nc = bacc.Bacc("TRN2", target_bir_lowering=False, debug=False,
                   num_devices=NCORES)

    zt_d = nc.dram_tensor("zt", (Z, BSC), f32r, kind="ExternalInput").ap()
    w1_d = nc.dram_tensor("w1", (512, 512), f32r, kind="ExternalInput").ap()
    wk_d = nc.dram_tensor("wk", (P, _WKW), f32r, kind="ExternalInput").ap()
    o_d = nc.dram_tensor("o", (BSC, 256), f32, kind="ExternalOutput").ap()

    lo_x, hi_x = float(b0), float(b2)
    lo_y, hi_y = float(b1), float(b3)

    with tile.TileContext(nc) as tc:
        with (
            tc.tile_pool(name="wp", bufs=1) as wp,
            tc.tile_pool(name="zp", bufs=2) as zp,
            tc.tile_pool(name="hp", bufs=2) as hp,
            tc.tile_pool(name="stg", bufs=3) as stg,
            tc.tile_pool(name="tmp", bufs=2) as tmp,
            tc.tile_pool(name="big", bufs=3, space="PSUM") as big,
            tc.tile_pool(name="ps5", bufs=2, space="PSUM") as ps5p,
        ):
            # ---- resident weights (w1 split per k; rest in one packed DMA) ----
            w1_sb = wp.tile([P, 4 * 512], f32r, tag="w1")
            w1v = w1_d.rearrange("(k p) m -> p k m", p=P)
            wk_sb = wp.tile([P, _WKW], f32r, tag="wk")

            def w1k(k):
                return w1_sb[:, k * 512:(k + 1) * 512]

            w2v = wk_sb[:, _W2O:_W2O + 1024]
            w3v = wk_sb[:, _W3O:_W3O + 256]
            w4v = wk_sb[:, _W4O:_W4O + 64]
            w5v = wk_sb[:, _W5O:_W5O + 256]
            b1v = wk_sb[:, _B1O:_B1O + 4].bitcast(f32)
            b2v = wk_sb[:, _B2O:_B2O + 2].bitcast(f32)
            b3v = wk_sb[:, _B3O:_B3O + 1].bitcast(f32)
            b4v = wk_sb[:, _B4O:_B4O + 1].bitcast(f32)

            for rep in range(reps):
              for n in range(BSC // NCH):
                # ---- load z chunk (feature-major, per-k split on chunk 0) ----
                zt_n = zp.tile([P, 4 * NCH], f32r, tag="zt")
                first = (rep == 0 and n == 0)
                if first:
                    for k in range(4):
                        nc.sync.dma_start(w1_sb[:, k * 512:(k + 1) * 512],
                                          w1v[:, k, :])
                        nc.sync.dma_start(
                            zt_n[:, k * NCH:(k + 1) * NCH],
                            zt_d[k * P:(k + 1) * P, n * NCH:(n + 1) * NCH])
                    nc.sync.dma_start(wk_sb[:], wk_d)
                else:
                    nc.sync.dma_start(
                        zt_n[:].rearrange("p (k c) -> p k c", k=4),
                        zt_d[:, n * NCH:(n + 1) * NCH]
                            .rearrange("(k p) c -> p k c", p=P))

                # ---- L1: 512 -> 512, k-outer in m-pair halves ----
                h1_n = hp.tile([P, 4 * NCH], f32r, tag="h1")
                for mh in range(2):
                    ps_a = big.tile([P, NCH], f32, tag="big")
                    ps_b = big.tile([P, NCH], f32, tag="big")
                    pss = [ps_a, ps_b]
                    for k in range(4):
                        for mi in range(2):
                            m = 2 * mh + mi
                            for hf in range(2):
                                nc.tensor.matmul(
                                    pss[mi][:, hf * 512:(hf + 1) * 512],
                                    w1k(k)[:, m * 128:(m + 1) * 128],
                                    zt_n[:, k * NCH + hf * 512:
                                         k * NCH + (hf + 1) * 512],
                                    start=(k == 0), stop=(k == 3))
                    for mi in range(2):
                        m = 2 * mh + mi
                        nc.scalar.activation(
                            h1_n[:, m * NCH:(m + 1) * NCH], pss[mi][:],
                            AF.Prelu, bias=b1v[:, m:m + 1], alpha=0.2)

                # ---- L2: 512 -> 256 ----
                h2_n = hp.tile([P, 2 * NCH], f32r, tag="h2")
                for m in range(2):
                    pst = big.tile([P, NCH], f32, tag="big")
                    for k in range(4):
                        for hf in range(2):
                            nc.tensor.matmul(
                                pst[:, hf * 512:(hf + 1) * 512],
                                w2v[:, k * 256 + m * 128:k * 256 + (m + 1) * 128],
                                h1_n[:, k * NCH + hf * 512:k * NCH + (hf + 1) * 512],
                                start=(k == 0), stop=(k == 3))
                    nc.scalar.activation(
                        h2_n[:, m * NCH:(m + 1) * NCH], pst[:],
                        AF.Prelu, bias=b2v[:, m:m + 1], alpha=0.2)

                # ---- L3: 256 -> 128 ----
                h3_n = hp.tile([P, NCH], f32r, tag="h3")
                pst = big.tile([P, NCH], f32, tag="big")
                for k in range(2):
                    for hf in range(2):
                        nc.tensor.matmul(
                            pst[:, hf * 512:(hf + 1) * 512],
                            w3v[:, k * 128:(k + 1) * 128],
                            h2_n[:, k * NCH + hf * 512:k * NCH + (hf + 1) * 512],
                            start=(k == 0), stop=(k == 1))
                nc.scalar.activation(h3_n[:], pst[:], AF.Prelu,
                                     bias=b3v[:, 0:1], alpha=0.2)

                # ---- L4: 128 -> 64 (plus ones row for L5 bias) ----
                h4_n = hp.tile([65, NCH], f32r, tag="h4")
                pst = big.tile([P, NCH], f32, tag="big")
                for hf in range(2):
                    nc.tensor.matmul(pst[0:64, hf * 512:(hf + 1) * 512],
                                     w4v[:], h3_n[:, hf * 512:(hf + 1) * 512],
                                     start=True, stop=True)
                nc.scalar.activation(h4_n[0:64, :], pst[0:64, :], AF.Prelu,
                                     bias=b4v[0:64, 0:1], alpha=0.2)
                nc.gpsimd.memset(h4_n[64:65, :].bitcast(f32), 1.0)

                # ---- L5 + QP per staging of 4 subtiles (512 batch rows) ----
                for sh in range(2):
                    p_sb = stg.tile([P, 1024], f32, tag="p")
                    o_sb = stg.tile([P, 1024], f32, tag="o")
                    for pair in range(2):
                        p5 = ps5p.tile([P, 512], f32, tag="l5")
                        for j in range(2):
                            sub = sh * 4 + pair * 2 + j
                            nc.tensor.matmul(
                                p5[:, j * 256:(j + 1) * 256],
                                h4_n[0:65, sub * P:(sub + 1) * P],
                                w5v[0:65, :], start=True, stop=True)
                        if pair == 0:
                            nc.vector.tensor_copy(
                                p_sb[:, pair * 512:(pair + 1) * 512], p5[:])
                        else:
                            nc.scalar.copy(
                                p_sb[:, pair * 512:(pair + 1) * 512], p5[:])

                    # QP solve (batch-major, immediates baked)
                    S = 4
                    pv = p_sb[:].rearrange("p (s o c) -> p s o c", s=S, o=NOBJ)
                    ov = o_sb[:].rearrange("p (s o c) -> p s o c", s=S, o=NOBJ)
                    if b0 == b1 and b2 == b3:
                        groups = [((0, 2), 2, lo_x, hi_x)]
                    else:
                        groups = [((0, 2), 1, lo_x, hi_x),
                                  ((1, 3), 1, lo_y, hi_y)]
                    for (cpp, cpg), cw, lo, hi in groups:
                        fd = S * NOBJ * cw
                        pp = pv[:, :, :, cpp:cpp + cw]
                        pg = pv[:, :, :, cpg:cpg + cw]

                        def tv(t):
                            return t[:, 0:fd].rearrange(
                                "p (s o c) -> p s o c", s=S, o=NOBJ)

                        rx = tmp.tile([P, fd], f32, tag="rx")
                        rg = tmp.tile([P, fd], f32, tag="rg")
                        s0 = tmp.tile([P, fd], f32, tag="s0")
                        u = tmp.tile([P, fd], f32, tag="u")
                        t1 = tmp.tile([P, fd], f32, tag="t1")
                        rxv, rgv, s0v, uv, t1v = map(tv, (rx, rg, s0, u, t1))
                        # rx = max(pp - lo, 0); rg = max(pg - 1, 0)
                        nc.vector.tensor_scalar(rxv, pp, lo, 0.0,
                                                Alu.subtract, Alu.max)
                        nc.vector.tensor_scalar(rgv, pg, 1.0, 0.0,
                                                Alu.subtract, Alu.max)
                        # s0 = (rx + (lo+1)) + rg ;  w = min(s0, hi)
                        nc.vector.scalar_tensor_tensor(
                            s0v, rxv, lo + 1.0, rgv, Alu.add, Alu.add)
                        nc.vector.tensor_scalar_min(
                            ov[:, :, :, cpg:cpg + cw], s0v, hi)
                        # u = (pp + hi) - pg ; u = min(0.5*u, hi-1)
                        nc.vector.scalar_tensor_tensor(
                            uv, pp, hi, pg, Alu.add, Alu.subtract)
                        nc.vector.tensor_scalar(uv, uv, 0.5, hi - 1.0,
                                                Alu.mult, Alu.min)
                        # x = max(min(rx + lo, u), lo)
                        nc.vector.scalar_tensor_tensor(
                            t1v, rxv, lo, uv, Alu.add, Alu.min)
                        nc.vector.tensor_scalar_max(
                            ov[:, :, :, cpp:cpp + cw], t1v, lo)

                    # ---- store staging -> DRAM (contiguous rows) ----
                    r0 = n * NCH + sh * 512
                    nc.sync.dma_start(
                        o_d[r0:r0 + 512, :].rearrange("(s p) f -> p s f", p=P),
                        o_sb[:].rearrange("p (s f) -> p s f", s=4))

    nc.compile()
    return nc


def _get_nc(b0, b1, b2, b3, reps=1):
    key = (b0, b1, b2, b3, reps)
    if key not in _cache:
        _cache[key] = _build(b0, b1, b2, b3, reps)
    return _cache[key]


def _prep_inputs(z, bounds, W1, c1, W2, c2, W3, c3, W4, c4, W5, c5):
    b = np.asarray(bounds, np.float32)
    W1m = np.ascontiguousarray(W1[:Z], np.float32)
    b1 = (np.asarray(c1, np.float32)
          + b @ np.asarray(W1[Z:], np.float32)).astype(np.float32)

    wk = np.zeros((P, _WKW), np.float32)
    wk[:, _W2O:_W2O + 1024] = (np.asarray(W2, np.float32)
                               .reshape(4, P, 256).transpose(1, 0, 2)
                               .reshape(P, 1024))
    wk[:, _W3O:_W3O + 256] = (np.asarray(W3, np.float32)
                              .reshape(2, P, 128).transpose(1, 0, 2)
                              .reshape(P, 256))
    wk[:, _W4O:_W4O + 64] = np.asarray(W4, np.float32)
    w5a = np.concatenate(
        [np.asarray(W5, np.float32), np.asarray(c5, np.float32)[None, :]], 0)
    wk[0:65, _W5O:_W5O + 256] = w5a
    wk[:, _B1O:_B1O + 4] = b1.reshape(4, P).T
    wk[:, _B2O:_B2O + 2] = np.asarray(c2, np.float32).reshape(2, P).T
    wk[:, _B3O] = np.asarray(c3, np.float32)
    wk[0:64, _B4O] = np.asarray(c4, np.float32)

    zT = np.ascontiguousarray(np.asarray(z, np.float32).T)
    common = {"w1": W1m, "wk": wk}
    in_maps = []
    for i in range(NCORES):
        m = dict(common)
        m["zt"] = np.ascontiguousarray(zT[:, i * BSC:(i + 1) * BSC])
        in_maps.append(m)
    return in_maps, (float(b[0]), float(b[1]), float(b[2]), float(b[3]))


def kernel(z, bounds, W1, c1, W2, c2, W3, c3, W4, c4, W5, c5):
    from concourse.bass_utils import run_bass_kernel_spmd

    in_maps, bvals = _prep_inputs(z, bounds, W1, c1, W2, c2, W3, c3,
                                  W4, c4, W5, c5)
    nc = _get_nc(*bvals)
    res = run_bass_kernel_spmd(nc, in_maps, core_ids=list(range(NCORES)))
    out = np.concatenate([r["o"] for r in res.results], axis=0)
    return out.reshape(BS, NOBJ, 4)


# revision 17
# speedup vs baseline: 487.6466x; 1.1210x over previous
"""Trainium2 Bass kernel for nn_CVX_Reasoning_Engine.

MLP (16384x512 -> 512 -> 256 -> 128 -> 64 -> 256) with LeakyReLU(0.2),
followed by a closed-form per-object/axis QP solve.

Strategy:
- Pure data parallel over 8 NeuronCores (2048 batch rows each).
- Host-side prep: fold `bounds` contribution of the concat into layer-1
  bias; transpose z so activations flow feature-major on-chip (no
  device transposes between layers); append the layer-5 bias as an
  extra ones-row of the last activation (K=65 matmul) so layer 5 exits
  batch-major, ready for the elementwise QP and a contiguous store.
- Matmuls run in float32r (full-rate fp32 mode, TF32-like precision).
- L1 is k-outer with per-k weight/input DMA splits so the PE starts
  after ~0.5MB of input instead of 2MB.
- Chunks of 1024 batch columns; activations are one ACT op per m-tile
  (bias + PReLU(0.2) fused, PSUM -> SBUF).
- QP closed form without branches (all on DVE, immediates baked in):
    x0 = max(pp, lo); g0 = max(pg, 1); s0 = x0 + g0
    w  = min(s0, hi)
    v  = 0.5*(pp - pg + hi)
    x  = max(min(min(x0, v), hi-1), lo)   # == reference KKT select
"""

import numpy as np

BS, Z, NOBJ = 16384, 512, 64
NCORES = 8
BSC = BS // NCORES            # 2048 batch rows per core
NCH = 1024                    # batch columns per chunk
P = 128

# packed-weight layout (per-partition float32 offsets)
_W2O, _W3O, _W4O, _W5O = 0, 1024, 1280, 1344
_B1O, _B2O, _B3O, _B4O = 1600, 1604, 1606, 1607
_WKW = 1608

_cache = {}


def _build(b0, b1, b2, b3, reps=1, chunks=(1024, 1024)):
    import concourse.tile as tile
    from concourse import bacc, mybir

    f32 = mybir.dt.float32
    f32r = mybir.dt.float32r
    AF = mybir.ActivationFunctionType
    Alu = mybir.AluOpType

    nc = bacc.Bacc("TRN2", target_bir_lowering=False, debug=False,
                   num_devices=NCORES)

    zt_d = nc.dram_tensor("zt", (Z, BSC), f32r, kind="ExternalInput").ap()
    w1_d = nc.dram_tensor("w1", (512, 512), f32r, kind="ExternalInput").ap()
    wk_d = nc.dram_tensor("wk", (P, _WKW), f32r, kind="ExternalInput").ap()
    o_d = nc.dram_tensor("o", (BSC, 256), f32, kind="ExternalOutput").ap()

    lo_x, hi_x = float(b0), float(b2)
    lo_y, hi_y = float(b1), float(b3)

    with tile.TileContext(nc) as tc:
        with (
            tc.tile_pool(name="wp", bufs=1) as wp,
            tc.tile_pool(name="zp", bufs=2) as zp,
            tc.tile_pool(name="hp", bufs=2) as hp,
            tc.tile_pool(name="stg", bufs=3) as stg,
            tc.tile_pool(name="tmp", bufs=2) as tmp,
            tc.tile_pool(name="big", bufs=3, space="PSUM") as big,
            tc.tile_pool(name="ps5", bufs=2, space="PSUM") as ps5p,
        ):
            # ---- resident weights (w1 split per k; rest in one packed DMA) ----
            w1_sb = wp.tile([P, 4 * 512], f32r, tag="w1")
            w1v = w1_d.rearrange("(k p) m -> p k m", p=P)
            wk_sb = wp.tile([P, _WKW], f32r, tag="wk")

            def w1k(k):
                return w1_sb[:, k * 512:(k + 1) * 512]

            w2v = wk_sb[:, _W2O:_W2O + 1024]
            w3v = wk_sb[:, _W3O:_W3O + 256]
            w4v = wk_sb[:, _W4O:_W4O + 64]
            w5v = wk_sb[:, _W5O:_W5O + 256]
            b1v = wk_sb[:, _B1O:_B1O + 4].bitcast(f32)
            b2v = wk_sb[:, _B2O:_B2O + 2].bitcast(f32)
            b3v = wk_sb[:, _B3O:_B3O + 1].bitcast(f32)
            b4v = wk_sb[:, _B4O:_B4O + 1].bitcast(f32)

            for rep in range(reps):
              col0 = 0
              for ci, W in enumerate(chunks):
                first = (rep == 0 and ci == 0)
                hfs = []
                off = 0
                while off < W:
                    hw = min(512, W - off)
                    hfs.append((off, hw))
                    off += hw

                # ---- load z chunk (feature-major, per-k split on chunk 0) ----
                zt_n = zp.tile([P, 4 * W], f32r, tag="zt")
                if first:
                    for k in range(4):
                        nc.sync.dma_start(w1_sb[:, k * 512:(k + 1) * 512],
                                          w1v[:, k, :])
                        nc.sync.dma_start(
                            zt_n[:, k * W:(k + 1) * W],
                            zt_d[k * P:(k + 1) * P, col0:col0 + W])
                    nc.sync.dma_start(wk_sb[:], wk_d)
                else:
                    nc.sync.dma_start(
                        zt_n[:].rearrange("p (k c) -> p k c", k=4),
                        zt_d[:, col0:col0 + W]
                            .rearrange("(k p) c -> p k c", p=P))

                # ---- L1: 512 -> 512 ----
                # chunk 0 head: k-outer over m0/m1 so the PE starts as soon
                # as the first 256KB of w1/z land; elsewhere per-m
                # k-contiguous to avoid PSUM-slot stalls.
                h1_n = hp.tile([P, 4 * W], f32r, tag="h1")

                def l1_act(m, pst, W=W, h1_n=h1_n):
                    nc.scalar.activation(
                        h1_n[:, m * W:(m + 1) * W], pst[:, 0:W],
                        AF.Prelu, bias=b1v[:, m:m + 1], alpha=0.2)

                if first:
                    ps_a = big.tile([P, W], f32, tag="big")
                    ps_b = big.tile([P, W], f32, tag="big")
                    pss = [ps_a, ps_b]
                    for k in range(4):
                        for mi in range(2):
                            for off, hw in hfs:
                                nc.tensor.matmul(
                                    pss[mi][:, off:off + hw],
                                    w1k(k)[:, mi * 128:(mi + 1) * 128],
                                    zt_n[:, k * W + off:k * W + off + hw],
                                    start=(k == 0), stop=(k == 3))
                    for mi in range(2):
                        l1_act(mi, pss[mi])
                    rest = range(2, 4)
                else:
                    rest = range(4)
                for m in rest:
                    pst = big.tile([P, W], f32, tag="big")
                    for k in range(4):
                        for off, hw in hfs:
                            nc.tensor.matmul(
                                pst[:, off:off + hw],
                                w1k(k)[:, m * 128:(m + 1) * 128],
                                zt_n[:, k * W + off:k * W + off + hw],
                                start=(k == 0), stop=(k == 3))
                    l1_act(m, pst)

                # ---- L2: 512 -> 256 ----
                h2_n = hp.tile([P, 2 * W], f32r, tag="h2")
                for m in range(2):
                    pst = big.tile([P, W], f32, tag="big")
                    for k in range(4):
                        for off, hw in hfs:
                            nc.tensor.matmul(
                                pst[:, off:off + hw],
                                w2v[:, k * 256 + m * 128:k * 256 + (m + 1) * 128],
                                h1_n[:, k * W + off:k * W + off + hw],
                                start=(k == 0), stop=(k == 3))
                    nc.scalar.activation(
                        h2_n[:, m * W:(m + 1) * W], pst[:, 0:W],
                        AF.Prelu, bias=b2v[:, m:m + 1], alpha=0.2)

                # ---- L3: 256 -> 128 ----
                h3_n = hp.tile([P, W], f32r, tag="h3")
                pst = big.tile([P, W], f32, tag="big")
                for k in range(2):
                    for off, hw in hfs:
                        nc.tensor.matmul(
                            pst[:, off:off + hw],
                            w3v[:, k * 128:(k + 1) * 128],
                            h2_n[:, k * W + off:k * W + off + hw],
                            start=(k == 0), stop=(k == 1))
                for off, hw in hfs:
                    nc.scalar.activation(
                        h3_n[:, off:off + hw], pst[:, off:off + hw],
                        AF.Prelu, bias=b3v[:, 0:1], alpha=0.2)

                # ---- L4: 128 -> 64 (plus ones row for L5 bias) ----
                h4_n = hp.tile([65, W], f32r, tag="h4")
                pst = big.tile([P, W], f32, tag="big")
                for off, hw in hfs:
                    nc.tensor.matmul(pst[0:64, off:off + hw],
                                     w4v[:], h3_n[:, off:off + hw],
                                     start=True, stop=True)
                for off, hw in hfs:
                    nc.scalar.activation(
                        h4_n[0:64, off:off + hw], pst[0:64, off:off + hw],
                        AF.Prelu, bias=b4v[0:64, 0:1], alpha=0.2)
                nc.gpsimd.memset(h4_n[64:65, :].bitcast(f32), 1.0)

                # ---- L5 + QP per staging of 2 subtiles (256 batch rows) ----
                nstg = W // 256
                for st in range(nstg):
                    last_stg = (ci == len(chunks) - 1 and st == nstg - 1)
                    p_sb = stg.tile([P, 512], f32, tag="p")
                    o_sb = stg.tile([P, 512], f32, tag="o")
                    p5 = ps5p.tile([P, 512], f32, tag="l5")
                    for j in range(2):
                        sub = st * 2 + j
                        nc.tensor.matmul(
                            p5[:, j * 256:(j + 1) * 256],
                            h4_n[0:65, sub * P:(sub + 1) * P],
                            w5v[0:65, :], start=True, stop=True)
                    if st % 2 == 0:
                        nc.vector.tensor_copy(p_sb[:], p5[:])
                    else:
                        nc.scalar.activation(p_sb[:], p5[:], AF.Prelu,
                                             alpha=1.0)

                    # QP solve (batch-major, immediates baked)
                    S = 2
                    pv = p_sb[:].rearrange("p (s o c) -> p s o c", s=S, o=NOBJ)
                    ov = o_sb[:].rearrange("p (s o c) -> p s o c", s=S, o=NOBJ)
                    if b0 == b1 and b2 == b3:
                        groups = [((0, 2), 2, lo_x, hi_x)]
                    else:
                        groups = [((0, 2), 1, lo_x, hi_x),
                                  ((1, 3), 1, lo_y, hi_y)]
                    for (cpp, cpg), cw, lo, hi in groups:
                        fd = S * NOBJ * cw
                        pp = pv[:, :, :, cpp:cpp + cw]
                        pg = pv[:, :, :, cpg:cpg + cw]

                        def tv(t, fd=fd, cw=cw):
                            return t[:, 0:fd].rearrange(
                                "p (s o c) -> p s o c", s=S, o=NOBJ)

                        g0 = tmp.tile([P, fd], f32, tag="g0")
                        s0 = tmp.tile([P, fd], f32, tag="s0")
                        u = tmp.tile([P, fd], f32, tag="u")
                        t1 = tmp.tile([P, fd], f32, tag="t1")
                        g0v, s0v, uv, t1v = map(tv, (g0, s0, u, t1))
                        # g0 = max(pg, 1); s0 = max(pp, lo) + g0; w = min(s0, hi)
                        nc.gpsimd.tensor_scalar_max(g0v, pg, 1.0)
                        nc.vector.scalar_tensor_tensor(
                            s0v, pp, lo, g0v, Alu.max, Alu.add)
                        nc.vector.tensor_scalar_min(
                            ov[:, :, :, cpg:cpg + cw], s0v, hi)
                        # u = (pp + hi) - pg ; scale+clip on Pool
                        nc.vector.scalar_tensor_tensor(
                            uv, pp, hi, pg, Alu.add, Alu.subtract)
                        nc.gpsimd.tensor_scalar(uv, uv, 0.5, hi - 1.0,
                                                Alu.mult, Alu.min)
                        # x = max(min(max(pp, lo), u), lo)
                        nc.vector.scalar_tensor_tensor(
                            t1v, pp, lo, uv, Alu.max, Alu.min)
                        if lo == 0.0 and last_stg:
                            nc.scalar.activation(
                                ov[:, :, :, cpp:cpp + cw], t1v, AF.Relu)
                        else:
                            nc.vector.tensor_scalar_max(
                                ov[:, :, :, cpp:cpp + cw], t1v, lo)

                    # ---- store staging -> DRAM (contiguous rows) ----
                    r0 = col0 + st * 256
                    nc.sync.dma_start(
                        o_d[r0:r0 + 256, :].rearrange("(s p) f -> p s f", p=P),
                        o_sb[:].rearrange("p (s f) -> p s f", s=2))
                col0 += W

    nc.compile()
    return nc


def _get_nc(b0, b1, b2, b3, reps=1, chunks=(1024, 1024)):
    key = (b0, b1, b2, b3, reps, tuple(chunks))
    if key not in _cache:
        _cache[key] = _build(b0, b1, b2, b3, reps, chunks)
    return _cache[key]


def _prep_inputs(z, bounds, W1, c1, W2, c2, W3, c3, W4, c4, W5, c5):
    b = np.asarray(bounds, np.float32)
    W1m = np.ascontiguousarray(W1[:Z], np.float32)
    b1 = (np.asarray(c1, np.float32)
          + b @ np.asarray(W1[Z:], np.float32)).astype(np.float32)

    wk = np.zeros((P, _WKW), np.float32)
    wk[:, _W2O:_W2O + 1024] = (np.asarray(W2, np.float32)
                               .reshape(4, P, 256).transpose(1, 0, 2)
                               .reshape(P, 1024))
    wk[:, _W3O:_W3O + 256] = (np.asarray(W3, np.float32)
                              .reshape(2, P, 128).transpose(1, 0, 2)
                              .reshape(P, 256))
    wk[:, _W4O:_W4O + 64] = np.asarray(W4, np.float32)
    w5a = np.concatenate(
        [np.asarray(W5, np.float32), np.asarray(c5, np.float32)[None, :]], 0)
    wk[0:65, _W5O:_W5O + 256] = w5a
    wk[:, _B1O:_B1O + 4] = b1.reshape(4, P).T
    wk[:, _B2O:_B2O + 2] = np.asarray(c2, np.float32).reshape(2, P).T
    wk[:, _B3O] = np.asarray(c3, np.float32)
    wk[0:64, _B4O] = np.asarray(c4, np.float32)

    zT = np.ascontiguousarray(np.asarray(z, np.float32).T)
    common = {"w1": W1m, "wk": wk}
    in_maps = []
    for i in range(NCORES):
        m = dict(common)
        m["zt"] = np.ascontiguousarray(zT[:, i * BSC:(i + 1) * BSC])
        in_maps.append(m)
    return in_maps, (float(b[0]), float(b[1]), float(b[2]), float(b[3]))


def kernel(z, bounds, W1, c1, W2, c2, W3, c3, W4, c4, W5, c5):
    from concourse.bass_utils import run_bass_kernel_spmd

    in_maps, bvals = _prep_inputs(z, bounds, W1, c1, W2, c2, W3, c3,
                                  W4, c4, W5, c5)
    nc = _get_nc(*bvals)
    res = run_bass_kernel_spmd(nc, in_maps, core_ids=list(range(NCORES)))
    out = np.concatenate([r["o"] for r in res.results], axis=0)
    return out.reshape(BS, NOBJ, 4)
